# revision 2
# baseline (speedup 1.0000x reference)
"""Chamfer loss kernel v2 for 8x TRN2 NeuronCores (Bass/Tile).

Strategy (data-parallel over batch, one batch per core):
  Host: kd-sort each cloud into 64 leaves of 128 points; per leaf select C
  candidate points of the other cloud by point-to-box LB distance; gather.
  Device: fp16 points => PE products exact; negated distance via K=5 matmul
     negD = 2 t.p - (p2h + p2l);  min dist = t2 - max(negD)
  Per row-tile: 3 matmuls (256 cols, psum-bank aligned) -> PSUM [128, C];
  paired-tile max-reduce on DVE straight from PSUM.
  res[128,1] = sum_tiles (t2 - negmax);  host: loss = res.sum() / (B*N).

Self-contained: only needs the concourse/axon environment, no sibling files.
"""
import numpy as np
import ml_dtypes
from contextlib import ExitStack

import jax
from jax.sharding import Mesh, PartitionSpec
from jax.experimental.shard_map import shard_map

import concourse.bacc as bacc
import concourse.tile as tile
import concourse.mybir as mybir
import concourse.bass as bass
from concourse.bass2jax import (
    _bass_exec_p,
    install_neuronx_cc_hook,
    partition_id_tensor,
)

N_CORES = 8
NPTS = 8192
C = 768                  # candidates per row-tile
NT = 128                 # row tiles (64 per orientation)
QC = NT * C // 128       # cands natural-layout q size (1024)
GRP = 8                  # tiles per DMA group in main loop
N_POOL = 58              # tiles whose reduce runs on GpSimd (rest on DVE)

F32 = mybir.dt.float32
FP16 = mybir.dt.float16
BF16 = mybir.dt.bfloat16
MAX = mybir.AluOpType.max
ADD = mybir.AluOpType.add
X = mybir.AxisListType.X

# evenly spread pool-assigned tiles
POOL_SET = frozenset(i for i in range(NT) if (i * N_POOL) // NT != ((i + 1) * N_POOL) // NT)


def build_nc(npts=NPTS, reps=1, hw_loop=True):
    assert npts == NPTS
    nc = bacc.Bacc("TRN2", target_bir_lowering=False, debug=False)
    rows_il = nc.dram_tensor("rows_il", [128, 128 * 3], F32, kind="ExternalInput")
    cands_il = nc.dram_tensor("cands_il", [128, QC * 3], F32, kind="ExternalInput")
    ident = nc.dram_tensor("ident", [128, 128], FP16, kind="ExternalInput")
    out = nc.dram_tensor("res", [128, 1], F32, kind="ExternalOutput")
    mov_dram = nc.dram_tensor("mov_scratch", [5, NT * C], FP16, kind="Internal")

    with tile.TileContext(nc) as tc, ExitStack() as ctx:
        sb = ctx.enter_context(tc.tile_pool(name="sb", bufs=1))

        # ---------------- persistent tiles ----------------
        wS = sb.tile([5, NT * 128], FP16, name="wS")       # stationary K-major
        t2buf = sb.tile([128, NT], F32, name="t2buf")      # |t~|^2 per (p, tile)
        mins_neg = sb.tile([128, NT], F32, name="minsneg")
        res = sb.tile([128, 1], F32, name="res")
        idt = sb.tile([128, 128], FP16, name="idt")
        nc.sync.dma_start(idt[:], ident.ap())

        # ---------------- setup: rows ----------------
        with tc.tile_pool(name="rsc", bufs=1) as rsc:
            rows_nat = rsc.tile([128, 128 * 3], F32)
            nc.sync.dma_start(rows_nat[:], rows_il.ap())
            rows16 = rsc.tile([128, 128 * 3], FP16)
            nc.vector.tensor_copy(rows16[:], rows_nat[:])
            r16v = rows16[:].rearrange("p (q d) -> p q d", d=3)
            # t2buf = sum of squares of fp16-rounded rows
            sqr = rsc.tile([128, 128 * 3], F32)
            sqv = sqr[:].rearrange("p (q d) -> p q d", d=3)
            nc.vector.tensor_mul(sqr[:], rows16[:], rows16[:])
            t2v = t2buf[:].rearrange("p (q d) -> p q d", d=1)
            nc.vector.tensor_add(t2v[:, :, 0:1], sqv[:, :, 0:1], sqv[:, :, 1:2])
            nc.vector.tensor_add(t2v[:, :, 0:1], t2v[:, :, 0:1], sqv[:, :, 2:3])
            # stationary natural assembly: [2x, 2y, 2z, -1, -1]
            S_rows = rsc.tile([128, 128 * 5], FP16)
            srv = S_rows[:].rearrange("p (q c) -> p q c", c=5)
            nc.vector.tensor_scalar_mul(srv[:, :, 0:3], r16v[:], 2.0)
            nc.vector.memset(srv[:, :, 3:5], -1.0)
            # transpose to K-major wS
            with tc.tile_pool(name="rps", bufs=2, space="PSUM") as rps:
                for b0 in range(0, 128, 16):
                    tp = rps.tile([5, 16 * 128], FP16)
                    for j in range(16):
                        nc.tensor.transpose(
                            tp[:, j * 128:(j + 1) * 128],
                            srv[:, b0 + j:b0 + j + 1, 0:5],
                            idt[:],
                        )
                    nc.vector.tensor_copy(wS[:, b0 * 128:(b0 + 16) * 128], tp[:])

        # ---------------- setup: candidates (2 halves, scratch reuse) ------
        HQ = QC // 2  # 512 blocks per half
        with tc.tile_pool(name="csc", bufs=1) as csc:
            c_nat = csc.tile([128, HQ * 3], F32)
            c16 = csc.tile([128, HQ * 3], FP16)
            sqc = csc.tile([128, HQ * 3], F32)
            p2 = csc.tile([128, HQ], F32)
            p2h32 = csc.tile([128, HQ], F32)
            S_c = csc.tile([128, HQ * 5], FP16)
            stag = [csc.tile([5, 16 * 128], FP16, name=f"stag{i}") for i in range(2)]
            for h in range(2):
                nc.sync.dma_start(c_nat[:], cands_il.ap()[:, h * HQ * 3:(h + 1) * HQ * 3])
                nc.vector.tensor_copy(c16[:], c_nat[:])
                c16v = c16[:].rearrange("p (q d) -> p q d", d=3)
                sqcv = sqc[:].rearrange("p (q d) -> p q d", d=3)
                nc.vector.tensor_mul(sqc[:], c16[:], c16[:])
                p2v = p2[:].rearrange("p (q d) -> p q d", d=1)
                nc.vector.tensor_add(p2v[:, :, 0:1], sqcv[:, :, 0:1], sqcv[:, :, 1:2])
                nc.vector.tensor_add(p2v[:, :, 0:1], p2v[:, :, 0:1], sqcv[:, :, 2:3])
                scv = S_c[:].rearrange("p (q c) -> p q c", c=5)
                nc.vector.tensor_copy(scv[:, :, 0:3], c16v[:])
                p2hv = scv[:, :, 3:4]
                nc.vector.tensor_copy(p2hv[:], p2v[:, :, 0:1])      # f32 -> fp16 (hi)
                p2h32v = p2h32[:].rearrange("p (q d) -> p q d", d=1)
                nc.vector.tensor_copy(p2h32v[:], p2hv[:])           # fp16 -> f32
                nc.vector.tensor_sub(scv[:, :, 4:5], p2v[:, :, 0:1], p2h32v[:])  # lo
                with tc.tile_pool(name="cps", bufs=2, space="PSUM") as cps:
                    for b0 in range(0, HQ, 16):
                        tp = cps.tile([5, 16 * 128], FP16)
                        for j in range(16):
                            nc.tensor.transpose(
                                tp[:, j * 128:(j + 1) * 128],
                                scv[:, b0 + j:b0 + j + 1, 0:5],
                                idt[:],
                            )
                        st = stag[(b0 // 16) % 2]
                        nc.vector.tensor_copy(st[:], tp[:])
                        g0 = (h * HQ + b0) * 128
                        nc.sync.dma_start(mov_dram.ap()[:, g0:g0 + 16 * 128], st[:])

        # ---------------- main rep loop ----------------
        ring_pool = ctx.enter_context(tc.tile_pool(name="ring", bufs=3))
        small = ctx.enter_context(tc.tile_pool(name="small", bufs=2))
        rep_ctx = ExitStack()
        with tc.tile_pool(name="pp", bufs=2, space="PSUM") as pp, rep_ctx:
            if reps > 1 and hw_loop:
                rep_ctx.enter_context(tc.For_i(0, reps, 1))
            n_unroll = reps if (reps > 1 and not hw_loop) else 1

            for _u in range(n_unroll):
              for g in range(NT // GRP):
                ring = ring_pool.tile([5, GRP * C], FP16)
                nc.sync.dma_start(
                    ring[:], mov_dram.ap()[:, g * GRP * C:(g + 1) * GRP * C])
                H = 256  # matmul chunk: divides the 512-f32 psum bank
                for j in range(0, GRP, 2):
                    t = g * GRP + j
                    pt = pp.tile([128, 2 * C], F32)
                    for k in range(2):
                        for c0 in range(0, C, H):
                            nc.tensor.matmul(
                                pt[:, k * C + c0:k * C + c0 + H],
                                wS[:, (t + k) * 128:(t + k + 1) * 128],
                                ring[:, (j + k) * C + c0:(j + k) * C + c0 + H],
                                start=True, stop=True,
                            )
                    nc.vector.tensor_reduce(
                        mins_neg[:, t:t + 2],
                        pt[:].rearrange("p (a f) -> p a f", a=2),
                        axis=X, op=MAX)
            # epilogue: res = sum(t2buf) - sum(mins_neg) per partition
            s1 = small.tile([128, 1], F32)
            s2 = small.tile([128, 1], F32)
            nc.vector.tensor_reduce(s1[:], t2buf[:], axis=X, op=ADD)
            nc.vector.tensor_reduce(s2[:], mins_neg[:], axis=X, op=ADD)
            nc.vector.tensor_sub(res[:], s1[:], s2[:])
            nc.sync.dma_start(out.ap(), res[:])

    nc.compile()
    return nc


# ----------------------------------------------------------------------
# Host-side preprocessing
# ----------------------------------------------------------------------
def kd_order(x, leaf=128):
    """Permutation putting kd-tree leaves of `leaf` points contiguous."""
    out = []

    def rec(ids):
        if len(ids) <= leaf:
            out.append(ids)
            return
        pts = x[ids]
        dim = int(np.argmax(pts.max(0) - pts.min(0)))
        half = len(ids) // 2
        part = np.argpartition(pts[:, dim], half)
        rec(ids[part[:half]])
        rec(ids[part[half:]])

    rec(np.arange(len(x)))
    return np.concatenate(out)


def prep_core(t, p):
    """One batch -> (rows_il [128,384] f32, cands_il [128, QC*3] f32)."""
    ot = kd_order(t)
    op = kd_order(p)
    ts = t[ot]
    ps = p[op]
    rows = np.concatenate([ts, ps], axis=0)             # [16384, 3]
    cands = np.empty((NT, C, 3), dtype=np.float32)
    for half, (rws, pool_pts) in enumerate(((ts, ps), (ps, ts))):
        tb = rws.reshape(64, 128, 3)
        lo = tb.min(1)[:, None, :]
        hi = tb.max(1)[:, None, :]
        g = np.maximum(0.0, np.maximum(lo - pool_pts[None], pool_pts[None] - hi))
        LB = (g * g).sum(-1)                            # [64, NPTS]
        sel = np.argpartition(LB, C, axis=1)[:, :C]     # [64, C]
        cands[half * 64:(half + 1) * 64] = pool_pts[sel]
    rows_il = np.ascontiguousarray(
        rows.reshape(128, 128, 3).transpose(1, 0, 2).reshape(128, 128 * 3))
    cands_il = np.ascontiguousarray(
        cands.reshape(NT * C // 128, 128, 3).transpose(1, 0, 2).reshape(128, QC * 3))
    return rows_il, cands_il


# ----------------------------------------------------------------------
# Runner with jit cache (same structure as baseline)
# ----------------------------------------------------------------------
_CACHE = {}


def _make_callable(nc, n_cores):
    install_neuronx_cc_hook()
    partition_name = nc.partition_id_tensor.name if nc.partition_id_tensor else None

    in_names, out_names, out_avals, zero_outs = [], [], [], []
    for alloc in nc.m.functions[0].allocations:
        if not isinstance(alloc, mybir.MemoryLocationSet):
            continue
        name = alloc.memorylocations[0].name
        if alloc.kind == "ExternalInput":
            if name != partition_name:
                in_names.append(name)
        elif alloc.kind == "ExternalOutput":
            out_names.append(name)
            shape = tuple(alloc.tensor_shape)
            dtype = mybir.dt.np(alloc.dtype)
            out_avals.append(jax.core.ShapedArray(shape, dtype))
            zero_outs.append(np.zeros(shape, dtype))
    n_params = len(in_names)
    n_outs = len(out_avals)
    all_in_names = list(in_names) + list(out_names)
    if partition_name is not None:
        all_in_names.append(partition_name)

    def _body(*args):
        operands = list(args)
        if partition_name is not None:
            operands.append(partition_id_tensor())
        outs = _bass_exec_p.bind(
            *operands,
            out_avals=tuple(out_avals),
            in_names=tuple(all_in_names),
            out_names=tuple(out_names),
            lowering_input_output_aliases=(),
            sim_require_finite=True,
            sim_require_nnan=True,
            nc=nc,
        )
        return tuple(outs)

    devices = jax.devices()[:n_cores]
    mesh = Mesh(np.asarray(devices), ("core",))
    in_specs = (PartitionSpec("core"),) * (n_params + n_outs)
    out_specs = (PartitionSpec("core"),) * n_outs
    fn = jax.jit(
        shard_map(_body, mesh=mesh, in_specs=in_specs, out_specs=out_specs,
                  check_rep=False),
        keep_unused=True,
    )
    return fn, in_names, out_names, out_avals, zero_outs


def get_runner(reps=1):
    key = ("runner", reps)
    if key not in _CACHE:
        nc = build_nc(NPTS, reps=reps)
        _CACHE[key] = _make_callable(nc, N_CORES)
    return _CACHE[key]


def make_inputs(pred, target):
    ident = np.eye(128, dtype=np.float16)
    per_core = {"rows_il": [], "cands_il": [], "ident": [ident] * N_CORES}
    for b in range(N_CORES):
        rows_il, cands_il = prep_core(
            np.asarray(target[b], dtype=np.float32),
            np.asarray(pred[b], dtype=np.float32))
        per_core["rows_il"].append(rows_il)
        per_core["cands_il"].append(cands_il)
    return per_core


def run_cores(per_core, reps=1):
    fn, in_names, out_names, out_avals, zero_outs = get_runner(reps)
    concat_in = [np.concatenate(per_core[name], axis=0) for name in in_names]
    concat_zero = [np.zeros((N_CORES * z.shape[0], *z.shape[1:]), z.dtype)
                   for z in zero_outs]
    outs = fn(*concat_in, *concat_zero)
    res = np.asarray(outs[out_names.index("res")]).reshape(N_CORES, 128, 1)
    return res


def kernel(pred, target):
    pred = np.asarray(pred, dtype=np.float32)
    target = np.asarray(target, dtype=np.float32)
    per_core = make_inputs(pred, target)
    res = run_cores(per_core)
    loss = res.astype(np.float64).sum() / float(pred.shape[0] * pred.shape[1])
    return np.float32(loss)
